# revision 1
# baseline (speedup 1.0000x reference)
"""Trainium2 Bass kernel for nn_Memory_7378753815338 (retrieval_knn).

Strategy (per sharding hint): shard the key/value memory bank across the 8
NeuronCores along memory_size (M=262144 -> 32768 per core). Each core:
  - computes q = normalize(x @ W.T + b) (query projection, replicated),
  - computes its local score block q @ keys_local.T with the PE array using
    an exact fp32 = f32r_hi + f32r_lo operand split (3 f32r passes at full
    PE streaming rate instead of fp32's 4 half-rate passes),
  - harvests per-row top-8 of every 1024-wide score chunk straight from PSUM
    with the DVE Max8 / MaxIndex8 instructions (32 chunks -> 256 local
    candidates per row, guaranteed to contain every element of the global
    top-256 that lives in this shard).
The host then all-gathers the 8x256 candidates per row and re-selects the
global top-256 (exactly the re-selection step from the sharding hint),
computes softmax / y_hat / hinge loss from them.
"""

import numpy as np

import concourse.bacc as bacc
import concourse.mybir as mybir
import concourse.tile as tile
from concourse.masks import make_identity
from concourse.bass_utils import run_bass_kernel_spmd

N_CORES = 8
B, D, M_TOTAL = 512, 256, 262144
N_LOCAL = M_TOTAL // N_CORES          # 32768
CHUNK_W = 1024
NCH = N_LOCAL // CHUNK_W              # 32
TOP_K = 256
TEMP = 1.0
MARGIN = 0.1

F32 = mybir.dt.float32
F32R = mybir.dt.float32r


def _build_kernel(psum_bufs=3, kbufs=3):
    n_rt = B // 128
    nc = bacc.Bacc(None)

    xT_d = nc.declare_dram_parameter("xT", [D, B], F32, isOutput=False)
    WT_d = nc.declare_dram_parameter("WT", [D, D], F32, isOutput=False)
    bias_d = nc.declare_dram_parameter("bias_bc", [128, D], F32, isOutput=False)
    khi_d = nc.declare_dram_parameter("keysT_hi", [D, N_LOCAL], F32R, isOutput=False)
    klo_d = nc.declare_dram_parameter("keysT_lo", [D, N_LOCAL], F32R, isOutput=False)
    vals_d = nc.declare_dram_parameter("vals", [n_rt, 128, NCH, 8], F32, isOutput=True)
    idxs_d = nc.declare_dram_parameter("idxs", [n_rt, 128, NCH, 8], mybir.dt.uint16,
                                       isOutput=True)

    with tile.TileContext(nc) as tc:
        with (
            tc.tile_pool(name="const", bufs=1) as const_pool,
            tc.tile_pool(name="kpool", bufs=kbufs) as kpool,
            tc.tile_pool(name="out", bufs=1) as out_pool,
            tc.tile_pool(name="psum", bufs=psum_bufs, space="PSUM") as psum,
            tc.tile_pool(name="psq", bufs=1, space="PSUM") as psq,
        ):
            # ---- load replicated inputs ----
            xT_sb = const_pool.tile([128, 2, B], F32)
            WT_sb = const_pool.tile([128, 2, D], F32)
            bias_sb = const_pool.tile([128, D], F32)
            nc.gpsimd.dma_start(xT_sb[:], xT_d.rearrange("(j p) m -> p j m", p=128))
            nc.gpsimd.dma_start(WT_sb[:], WT_d.rearrange("(j p) m -> p j m", p=128))
            nc.gpsimd.dma_start(bias_sb[:], bias_d[:])

            # ---- q = normalize(x @ W.T + b), computed in [b, d] layout ----
            q_bd = const_pool.tile([128, n_rt, D], F32)
            norms = const_pool.tile([128, n_rt], F32)
            sq = const_pool.tile([128, n_rt, D], F32)
            for rt in range(n_rt):
                qacc = psq.tile([128, D], F32, tag="qacc")
                for j in range(2):
                    nc.tensor.matmul(qacc[:], xT_sb[:, j, rt * 128:(rt + 1) * 128],
                                     WT_sb[:, j, :], start=(j == 0), stop=(j == 1))
                nc.vector.tensor_add(q_bd[:, rt, :], qacc[:], bias_sb[:])
                nc.vector.tensor_mul(sq[:, rt, :], q_bd[:, rt, :], q_bd[:, rt, :])
                nc.vector.reduce_sum(norms[:, rt:rt + 1], sq[:, rt, :],
                                     axis=mybir.AxisListType.X)
            rsq = const_pool.tile([128, n_rt], F32)
            nc.scalar.sqrt(rsq[:], norms[:])
            nc.vector.reciprocal(norms[:], rsq[:])
            for rt in range(n_rt):
                nc.vector.tensor_scalar_mul(q_bd[:, rt, :], q_bd[:, rt, :],
                                            norms[:, rt:rt + 1])

            # ---- transpose q -> qT [d, b] via PE transpose ----
            ident = const_pool.tile([128, 128], F32)
            make_identity(nc, ident[:])
            qT_sb = const_pool.tile([128, 2, B], F32)
            for j in range(2):
                for rt in range(n_rt):
                    tacc = psq.tile([128, 128], F32, tag="tacc")
                    nc.tensor.transpose(tacc[:], q_bd[:, rt, j * 128:(j + 1) * 128],
                                        ident[:])
                    nc.vector.tensor_copy(qT_sb[:, j, rt * 128:(rt + 1) * 128], tacc[:])

            # exact q = qhi + qlo with both f32r-representable
            qhi = const_pool.tile([128, 2, B], F32R)
            qlo = const_pool.tile([128, 2, B], F32R)
            nc.vector.tensor_copy(qhi[:], qT_sb[:])
            nc.vector.tensor_sub(qlo[:], qT_sb[:], qhi[:].bitcast(F32))

            # ---- stream keysT chunks: 3 f32r passes per chunk, harvest top-8 ----
            vals_sb = out_pool.tile([128, n_rt, NCH, 8], F32)
            idxs_sb = out_pool.tile([128, n_rt, NCH, 8], mybir.dt.uint16)

            for ci in range(NCH):
                cs = slice(ci * CHUNK_W, (ci + 1) * CHUNK_W)
                kt_hi = kpool.tile([128, 2, CHUNK_W], F32R, tag="kthi")
                kt_lo = kpool.tile([128, 2, CHUNK_W], F32R, tag="ktlo")
                nc.gpsimd.dma_start(kt_hi[:], khi_d.rearrange(
                    "(j p) n -> p j n", p=128)[:, :, cs])
                nc.gpsimd.dma_start(kt_lo[:], klo_d.rearrange(
                    "(j p) n -> p j n", p=128)[:, :, cs])
                for rt in range(n_rt):
                    acc = psum.tile([128, CHUNK_W], F32, tag="acc")
                    rs = slice(rt * 128, (rt + 1) * 128)
                    for s0 in range(0, CHUNK_W, 512):
                        ss = slice(s0, s0 + 512)
                        first = True
                        for j in range(2):
                            nc.tensor.matmul(acc[:, ss], qhi[:, j, rs], kt_hi[:, j, ss],
                                             start=first, stop=False)
                            first = False
                            nc.tensor.matmul(acc[:, ss], qhi[:, j, rs], kt_lo[:, j, ss],
                                             start=False, stop=False)
                            nc.tensor.matmul(acc[:, ss], qlo[:, j, rs], kt_hi[:, j, ss],
                                             start=False, stop=(j == 1))
                    nc.vector.max(out=vals_sb[:, rt, ci, :], in_=acc[:])
                    nc.vector.max_index(out=idxs_sb[:, rt, ci, :],
                                        in_max=vals_sb[:, rt, ci, :], in_values=acc[:])

            nc.gpsimd.dma_start(vals_d.rearrange("r p c e -> p r c e"), vals_sb[:])
            nc.gpsimd.dma_start(idxs_d.rearrange("r p c e -> p r c e"), idxs_sb[:])

    nc.compile()
    return nc


_NC_CACHE = None


def _get_nc():
    global _NC_CACHE
    if _NC_CACHE is None:
        _NC_CACHE = _build_kernel()
    return _NC_CACHE


def _split_f32r(a):
    """fp32 = hi + lo with both exactly representable in f32r (12-bit
    significand): hi truncates the mantissa's low 12 bits, lo keeps them."""
    bits = a.view(np.uint32)
    hi = (bits & np.uint32(0xFFFFF000)).view(np.float32)
    lo = a - hi
    return hi, lo


def kernel(x, y, keys, values, W, b):
    x = np.asarray(x, dtype=np.float32)
    y = np.asarray(y)
    keys = np.asarray(keys, dtype=np.float32)
    values = np.asarray(values)
    W = np.asarray(W, dtype=np.float32)
    b = np.asarray(b, dtype=np.float32)

    xT = np.ascontiguousarray(x.T)
    WT = np.ascontiguousarray(W.T)
    bias_bc = np.broadcast_to(b, (128, D)).copy()

    in_maps = []
    for c in range(N_CORES):
        shard = keys[c * N_LOCAL:(c + 1) * N_LOCAL, :]
        keysT = np.ascontiguousarray(shard.T)
        khi, klo = _split_f32r(keysT)
        in_maps.append({
            "xT": xT, "WT": WT, "bias_bc": bias_bc,
            "keysT_hi": khi, "keysT_lo": klo,
        })

    nc = _get_nc()
    results = run_bass_kernel_spmd(nc, in_maps, list(range(N_CORES))).results

    # ---- all-gather candidates & re-select global top-k on host ----
    n_cand = NCH * 8                       # 256 per core per row
    vals = np.empty((B, N_CORES * n_cand), np.float32)
    gidx = np.empty((B, N_CORES * n_cand), np.int64)
    chunk_base = (np.arange(NCH, dtype=np.int64)[:, None] * CHUNK_W)  # [NCH, 1]
    for c in range(N_CORES):
        v = results[c]["vals"].reshape(B, NCH, 8)
        ix = results[c]["idxs"].reshape(B, NCH, 8).astype(np.int64)
        g = c * N_LOCAL + chunk_base[None, :, :] + ix
        vals[:, c * n_cand:(c + 1) * n_cand] = v.reshape(B, n_cand)
        gidx[:, c * n_cand:(c + 1) * n_cand] = g.reshape(B, n_cand)

    # top-256 per row; ties resolved toward lower global index like lax.top_k.
    part = np.argpartition(-vals, TOP_K - 1, axis=1)[:, :TOP_K]
    pv = np.take_along_axis(vals, part, axis=1)
    pg = np.take_along_axis(gidx, part, axis=1)
    # order by (-value, index): sort by index first (stable), then by -value
    o1 = np.argsort(pg, axis=1, kind="stable")
    pv = np.take_along_axis(pv, o1, axis=1)
    pg = np.take_along_axis(pg, o1, axis=1)
    o2 = np.argsort(-pv, axis=1, kind="stable")
    cos = np.take_along_axis(pv, o2, axis=1)            # [B, 256] sorted desc
    idx = np.take_along_axis(pg, o2, axis=1)            # [B, 256]

    # exact rescore of the leading candidates to pick the true argmax for
    # y_hat (device scores carry ~2e-6 absolute error from the f32r path)
    q = x.astype(np.float64) @ W.astype(np.float64).T + b.astype(np.float64)
    q /= np.linalg.norm(q, axis=1, keepdims=True)
    r = 8
    head_keys = keys[idx[:, :r], :].astype(np.float64)  # [B, r, D]
    head_scores = np.einsum("bd,brd->br", q, head_keys)
    best = np.argmax(head_scores, axis=1)               # exact argmax among top-8
    top1 = idx[np.arange(B), best]
    y_hat = values[top1]                                # [B, 1], dtype preserved

    # softmax over TEMP * cos (float32, matching jax.nn.softmax)
    z = (TEMP * cos).astype(np.float32)
    z = z - z.max(axis=1, keepdims=True)
    ez = np.exp(z)
    softmax_score = ez / ez.sum(axis=1, keepdims=True)

    # memory hinge loss
    topk_vals = values[idx]                             # [B, 256, 1]
    correct = (topk_vals == y[:, None, :]).squeeze(-1).astype(np.float32)
    pos = np.max(cos * correct, axis=1, keepdims=True)
    neg = np.max(cos * (1.0 - correct), axis=1, keepdims=True)
    has_pos = (correct.sum(axis=1) > 0).astype(np.float32)
    pos = pos * has_pos[:, None]
    loss = np.mean(np.maximum(neg - pos + MARGIN, 0.0).astype(np.float32))

    return (y_hat, softmax_score.astype(np.float32), np.float32(loss))


# revision 2
# speedup vs baseline: 1.0730x; 1.0730x over previous
"""Trainium2 Bass kernel for nn_Memory_7378753815338 (retrieval_knn).

Strategy (per sharding hint): shard the key/value memory bank across the 8
NeuronCores along memory_size (M=262144 -> 32768 per core). Each core:
  - computes q = normalize(x @ W.T + b) (query projection, replicated),
  - computes its local score block q @ keys_local.T on the PE array in f32r
    (fp32 with 12-bit significand, full streaming rate; absolute score error
    ~2e-5 which only matters at selection boundaries -- the host re-scores
    the leading candidates exactly before emitting y_hat),
  - harvests per-row top-8 of every 1024-wide score chunk straight from PSUM
    with the DVE Max8 / MaxIndex8 instructions (32 chunks -> 256 local
    candidates per row, guaranteed to contain every element of the global
    top-256 that lives in this shard).
The host then all-gathers the 8x256 candidates per row and re-selects the
global top-256 (exactly the re-selection step from the sharding hint),
computes softmax / y_hat / hinge loss from them.
"""

import numpy as np

import concourse.bacc as bacc
import concourse.mybir as mybir
import concourse.tile as tile
from concourse.masks import make_identity
from concourse.bass_utils import run_bass_kernel_spmd

N_CORES = 8
B, D, M_TOTAL = 512, 256, 262144
N_LOCAL = M_TOTAL // N_CORES          # 32768
CHUNK_W = 1024
NCH = N_LOCAL // CHUNK_W              # 32
TOP_K = 256
TEMP = 1.0
MARGIN = 0.1

F32 = mybir.dt.float32
F32R = mybir.dt.float32r


def _build_kernel(psum_bufs=3, kbufs=3):
    n_rt = B // 128
    nc = bacc.Bacc(None)

    xT_d = nc.declare_dram_parameter("xT", [D, B], F32, isOutput=False)
    WT_d = nc.declare_dram_parameter("WT", [D, D], F32, isOutput=False)
    bias_d = nc.declare_dram_parameter("bias_bc", [128, D], F32, isOutput=False)
    khi_d = nc.declare_dram_parameter("keysT_hi", [D, N_LOCAL], F32R, isOutput=False)
    vals_d = nc.declare_dram_parameter("vals", [n_rt, 128, NCH, 8], F32, isOutput=True)
    idxs_d = nc.declare_dram_parameter("idxs", [n_rt, 128, NCH, 8], mybir.dt.uint16,
                                       isOutput=True)

    with tile.TileContext(nc) as tc:
        with (
            tc.tile_pool(name="const", bufs=1) as const_pool,
            tc.tile_pool(name="kpool", bufs=kbufs) as kpool,
            tc.tile_pool(name="out", bufs=1) as out_pool,
            tc.tile_pool(name="psum", bufs=psum_bufs, space="PSUM") as psum,
            tc.tile_pool(name="psq", bufs=1, space="PSUM") as psq,
        ):
            # ---- load replicated inputs ----
            xT_sb = const_pool.tile([128, 2, B], F32)
            WT_sb = const_pool.tile([128, 2, D], F32)
            bias_sb = const_pool.tile([128, D], F32)
            nc.gpsimd.dma_start(xT_sb[:], xT_d.rearrange("(j p) m -> p j m", p=128))
            nc.gpsimd.dma_start(WT_sb[:], WT_d.rearrange("(j p) m -> p j m", p=128))
            nc.gpsimd.dma_start(bias_sb[:], bias_d[:])

            # ---- q = normalize(x @ W.T + b), computed in [b, d] layout ----
            q_bd = const_pool.tile([128, n_rt, D], F32)
            norms = const_pool.tile([128, n_rt], F32)
            sq = const_pool.tile([128, n_rt, D], F32)
            for rt in range(n_rt):
                qacc = psq.tile([128, D], F32, tag="qacc")
                for j in range(2):
                    nc.tensor.matmul(qacc[:], xT_sb[:, j, rt * 128:(rt + 1) * 128],
                                     WT_sb[:, j, :], start=(j == 0), stop=(j == 1))
                nc.vector.tensor_add(q_bd[:, rt, :], qacc[:], bias_sb[:])
                nc.vector.tensor_mul(sq[:, rt, :], q_bd[:, rt, :], q_bd[:, rt, :])
                nc.vector.reduce_sum(norms[:, rt:rt + 1], sq[:, rt, :],
                                     axis=mybir.AxisListType.X)
            rsq = const_pool.tile([128, n_rt], F32)
            nc.scalar.sqrt(rsq[:], norms[:])
            nc.vector.reciprocal(norms[:], rsq[:])
            for rt in range(n_rt):
                nc.vector.tensor_scalar_mul(q_bd[:, rt, :], q_bd[:, rt, :],
                                            norms[:, rt:rt + 1])

            # ---- transpose q -> qT [d, b] via PE transpose ----
            ident = const_pool.tile([128, 128], F32)
            make_identity(nc, ident[:])
            qT_sb = const_pool.tile([128, 2, B], F32)
            for j in range(2):
                for rt in range(n_rt):
                    tacc = psq.tile([128, 128], F32, tag="tacc")
                    nc.tensor.transpose(tacc[:], q_bd[:, rt, j * 128:(j + 1) * 128],
                                        ident[:])
                    nc.vector.tensor_copy(qT_sb[:, j, rt * 128:(rt + 1) * 128], tacc[:])

            # f32r-rounded q (12-bit significand)
            qhi = const_pool.tile([128, 2, B], F32R)
            nc.vector.tensor_copy(qhi[:], qT_sb[:])

            # ---- stream keysT chunks: 3 f32r passes per chunk, harvest top-8 ----
            vals_sb = out_pool.tile([128, n_rt, NCH, 8], F32)
            idxs_sb = out_pool.tile([128, n_rt, NCH, 8], mybir.dt.uint16)

            for ci in range(NCH):
                cs = slice(ci * CHUNK_W, (ci + 1) * CHUNK_W)
                kt_hi = kpool.tile([128, 2, CHUNK_W], F32R, tag="kthi")
                nc.gpsimd.dma_start(kt_hi[:], khi_d.rearrange(
                    "(j p) n -> p j n", p=128)[:, :, cs])
                for rt in range(n_rt):
                    acc = psum.tile([128, CHUNK_W], F32, tag="acc")
                    rs = slice(rt * 128, (rt + 1) * 128)
                    for j in range(2):
                        for s0 in range(0, CHUNK_W, 512):
                            ss = slice(s0, s0 + 512)
                            nc.tensor.matmul(acc[:, ss], qhi[:, j, rs], kt_hi[:, j, ss],
                                             start=(j == 0), stop=(j == 1))
                    nc.vector.max(out=vals_sb[:, rt, ci, :], in_=acc[:])
                    nc.vector.max_index(out=idxs_sb[:, rt, ci, :],
                                        in_max=vals_sb[:, rt, ci, :], in_values=acc[:])

            nc.gpsimd.dma_start(vals_d.rearrange("r p c e -> p r c e"), vals_sb[:])
            nc.gpsimd.dma_start(idxs_d.rearrange("r p c e -> p r c e"), idxs_sb[:])

    nc.compile()
    return nc


_NC_CACHE = None


def _get_nc():
    global _NC_CACHE
    if _NC_CACHE is None:
        _NC_CACHE = _build_kernel()
    return _NC_CACHE


def _split_f32r(a):
    """fp32 = hi + lo with both exactly representable in f32r (12-bit
    significand): hi truncates the mantissa's low 12 bits, lo keeps them."""
    bits = a.view(np.uint32)
    hi = (bits & np.uint32(0xFFFFF000)).view(np.float32)
    lo = a - hi
    return hi, lo


def kernel(x, y, keys, values, W, b):
    x = np.asarray(x, dtype=np.float32)
    y = np.asarray(y)
    keys = np.asarray(keys, dtype=np.float32)
    values = np.asarray(values)
    W = np.asarray(W, dtype=np.float32)
    b = np.asarray(b, dtype=np.float32)

    xT = np.ascontiguousarray(x.T)
    WT = np.ascontiguousarray(W.T)
    bias_bc = np.broadcast_to(b, (128, D)).copy()

    in_maps = []
    for c in range(N_CORES):
        shard = keys[c * N_LOCAL:(c + 1) * N_LOCAL, :]
        keysT = np.ascontiguousarray(shard.T)
        khi, _ = _split_f32r(keysT)
        in_maps.append({
            "xT": xT, "WT": WT, "bias_bc": bias_bc, "keysT_hi": khi,
        })

    nc = _get_nc()
    results = run_bass_kernel_spmd(nc, in_maps, list(range(N_CORES))).results

    # ---- all-gather candidates & re-select global top-k on host ----
    n_cand = NCH * 8                       # 256 per core per row
    vals = np.empty((B, N_CORES * n_cand), np.float32)
    gidx = np.empty((B, N_CORES * n_cand), np.int64)
    chunk_base = (np.arange(NCH, dtype=np.int64)[:, None] * CHUNK_W)  # [NCH, 1]
    for c in range(N_CORES):
        v = results[c]["vals"].reshape(B, NCH, 8)
        ix = results[c]["idxs"].reshape(B, NCH, 8).astype(np.int64)
        g = c * N_LOCAL + chunk_base[None, :, :] + ix
        vals[:, c * n_cand:(c + 1) * n_cand] = v.reshape(B, n_cand)
        gidx[:, c * n_cand:(c + 1) * n_cand] = g.reshape(B, n_cand)

    # top-256 per row; ties resolved toward lower global index like lax.top_k.
    part = np.argpartition(-vals, TOP_K - 1, axis=1)[:, :TOP_K]
    pv = np.take_along_axis(vals, part, axis=1)
    pg = np.take_along_axis(gidx, part, axis=1)
    # order by (-value, index): sort by index first (stable), then by -value
    o1 = np.argsort(pg, axis=1, kind="stable")
    pv = np.take_along_axis(pv, o1, axis=1)
    pg = np.take_along_axis(pg, o1, axis=1)
    o2 = np.argsort(-pv, axis=1, kind="stable")
    cos = np.take_along_axis(pv, o2, axis=1)            # [B, 256] sorted desc
    idx = np.take_along_axis(pg, o2, axis=1)            # [B, 256]

    # exact rescore of the leading candidates to pick the true argmax for
    # y_hat (device scores carry ~2e-6 absolute error from the f32r path)
    q = x.astype(np.float64) @ W.astype(np.float64).T + b.astype(np.float64)
    q /= np.linalg.norm(q, axis=1, keepdims=True)
    r = 8
    head_keys = keys[idx[:, :r], :].astype(np.float64)  # [B, r, D]
    head_scores = np.einsum("bd,brd->br", q, head_keys)
    best = np.argmax(head_scores, axis=1)               # exact argmax among top-8
    top1 = idx[np.arange(B), best]
    y_hat = values[top1]                                # [B, 1], dtype preserved

    # softmax over TEMP * cos (float32, matching jax.nn.softmax)
    z = (TEMP * cos).astype(np.float32)
    z = z - z.max(axis=1, keepdims=True)
    ez = np.exp(z)
    softmax_score = ez / ez.sum(axis=1, keepdims=True)

    # memory hinge loss
    topk_vals = values[idx]                             # [B, 256, 1]
    correct = (topk_vals == y[:, None, :]).squeeze(-1).astype(np.float32)
    pos = np.max(cos * correct, axis=1, keepdims=True)
    neg = np.max(cos * (1.0 - correct), axis=1, keepdims=True)
    has_pos = (correct.sum(axis=1) > 0).astype(np.float32)
    pos = pos * has_pos[:, None]
    loss = np.mean(np.maximum(neg - pos + MARGIN, 0.0).astype(np.float32))

    return (y_hat, softmax_score.astype(np.float32), np.float32(loss))


# revision 3
# speedup vs baseline: 1.2718x; 1.1853x over previous
"""Trainium2 Bass kernel for nn_Memory_7378753815338 (retrieval_knn).

Strategy (per sharding hint): shard the key/value memory bank across the 8
NeuronCores along memory_size (M=262144 -> 32768 per core). Each core:
  - computes q = normalize(x @ W.T + b) (query projection, replicated),
  - computes its local score block q @ keys_local.T on the PE array in f32r
    (fp32 with 12-bit significand, full streaming rate; absolute score error
    ~2e-5 which only matters at selection boundaries -- the host re-scores
    the leading candidates exactly before emitting y_hat),
  - harvests per-row top-8 of every 1024-wide score chunk straight from PSUM
    with the DVE Max8 / MaxIndex8 instructions (32 chunks -> 256 local
    candidates per row, guaranteed to contain every element of the global
    top-256 that lives in this shard).
The host then all-gathers the 8x256 candidates per row and re-selects the
global top-256 (exactly the re-selection step from the sharding hint),
computes softmax / y_hat / hinge loss from them.
"""

import numpy as np

import concourse.bacc as bacc
import concourse.mybir as mybir
import concourse.tile as tile
from concourse.masks import make_identity
from concourse.bass_utils import run_bass_kernel_spmd

N_CORES = 8
B, D, M_TOTAL = 512, 256, 262144
N_LOCAL = M_TOTAL // N_CORES          # 32768
CHUNK_W = 1024
NCH = N_LOCAL // CHUNK_W              # 32
TOP_K = 256
TEMP = 1.0
MARGIN = 0.1

F32 = mybir.dt.float32
F32R = mybir.dt.float32r


def _build_kernel(psum_bufs=3, kbufs=3, repeat=1):
    """repeat>1 duplicates the main scoring loop in-NEFF (used only by
    test.py to measure per-iteration hardware time via wall-clock slope)."""
    n_rt = B // 128
    nc = bacc.Bacc(None)

    xT_d = nc.declare_dram_parameter("xT", [D, B], F32, isOutput=False)
    WT_d = nc.declare_dram_parameter("WT", [D, D], F32, isOutput=False)
    bias_d = nc.declare_dram_parameter("bias_bc", [128, D], F32, isOutput=False)
    khi_d = nc.declare_dram_parameter("keysT_hi", [D, N_LOCAL], F32R, isOutput=False)
    vals_d = nc.declare_dram_parameter("vals", [n_rt, 128, NCH, 8], F32, isOutput=True)
    idxs_d = nc.declare_dram_parameter("idxs", [n_rt, 128, NCH, 8], mybir.dt.uint16,
                                       isOutput=True)

    with tile.TileContext(nc) as tc:
        with (
            tc.tile_pool(name="const", bufs=1) as const_pool,
            tc.tile_pool(name="kpool", bufs=kbufs) as kpool,
            tc.tile_pool(name="out", bufs=1) as out_pool,
            tc.tile_pool(name="psum", bufs=psum_bufs, space="PSUM") as psum,
            tc.tile_pool(name="psq", bufs=1, space="PSUM") as psq,
        ):
            # ---- load replicated inputs ----
            xT_sb = const_pool.tile([128, 2, B], F32)
            WT_sb = const_pool.tile([128, 2, D], F32)
            bias_sb = const_pool.tile([128, D], F32)
            nc.gpsimd.dma_start(xT_sb[:], xT_d.rearrange("(j p) m -> p j m", p=128))
            nc.gpsimd.dma_start(WT_sb[:], WT_d.rearrange("(j p) m -> p j m", p=128))
            nc.gpsimd.dma_start(bias_sb[:], bias_d[:])

            # ---- q = normalize(x @ W.T + b), computed in [b, d] layout ----
            q_bd = const_pool.tile([128, n_rt, D], F32)
            norms = const_pool.tile([128, n_rt], F32)
            sq = const_pool.tile([128, n_rt, D], F32)
            for rt in range(n_rt):
                qacc = psq.tile([128, D], F32, tag="qacc")
                for j in range(2):
                    nc.tensor.matmul(qacc[:], xT_sb[:, j, rt * 128:(rt + 1) * 128],
                                     WT_sb[:, j, :], start=(j == 0), stop=(j == 1))
                nc.vector.tensor_add(q_bd[:, rt, :], qacc[:], bias_sb[:])
                nc.vector.tensor_mul(sq[:, rt, :], q_bd[:, rt, :], q_bd[:, rt, :])
                nc.vector.reduce_sum(norms[:, rt:rt + 1], sq[:, rt, :],
                                     axis=mybir.AxisListType.X)
            rsq = const_pool.tile([128, n_rt], F32)
            nc.scalar.sqrt(rsq[:], norms[:])
            nc.vector.reciprocal(norms[:], rsq[:])
            for rt in range(n_rt):
                nc.vector.tensor_scalar_mul(q_bd[:, rt, :], q_bd[:, rt, :],
                                            norms[:, rt:rt + 1])

            # ---- transpose q -> qT [d, b] via PE transpose ----
            ident = const_pool.tile([128, 128], F32)
            make_identity(nc, ident[:])
            qT_sb = const_pool.tile([128, 2, B], F32)
            for j in range(2):
                for rt in range(n_rt):
                    tacc = psq.tile([128, 128], F32, tag="tacc")
                    nc.tensor.transpose(tacc[:], q_bd[:, rt, j * 128:(j + 1) * 128],
                                        ident[:])
                    nc.vector.tensor_copy(qT_sb[:, j, rt * 128:(rt + 1) * 128], tacc[:])

            # f32r-rounded q (12-bit significand)
            qhi = const_pool.tile([128, 2, B], F32R)
            nc.vector.tensor_copy(qhi[:], qT_sb[:])

            # ---- stream keysT chunks: 1 f32r pass per chunk, harvest top-8 ----
            vals_sb = out_pool.tile([128, n_rt, NCH, 8], F32)
            idxs_sb = out_pool.tile([128, n_rt, NCH, 8], mybir.dt.uint16)

            for ci in [c for _ in range(repeat) for c in range(NCH)]:
                cs = slice(ci * CHUNK_W, (ci + 1) * CHUNK_W)
                kt_hi = kpool.tile([128, 2, CHUNK_W], F32R, tag="kthi")
                nc.gpsimd.dma_start(kt_hi[:], khi_d.rearrange(
                    "(j p) n -> p j n", p=128)[:, :, cs])
                for rt in range(n_rt):
                    acc = psum.tile([128, CHUNK_W], F32, tag="acc")
                    rs = slice(rt * 128, (rt + 1) * 128)
                    for j in range(2):
                        for s0 in range(0, CHUNK_W, 512):
                            ss = slice(s0, s0 + 512)
                            nc.tensor.matmul(acc[:, ss], qhi[:, j, rs], kt_hi[:, j, ss],
                                             start=(j == 0), stop=(j == 1))
                    nc.vector.max(out=vals_sb[:, rt, ci, :], in_=acc[:])
                    nc.vector.max_index(out=idxs_sb[:, rt, ci, :],
                                        in_max=vals_sb[:, rt, ci, :], in_values=acc[:])

            nc.gpsimd.dma_start(vals_d.rearrange("r p c e -> p r c e"), vals_sb[:])
            nc.gpsimd.dma_start(idxs_d.rearrange("r p c e -> p r c e"), idxs_sb[:])

    nc.compile()
    return nc


_NC_CACHE = None


def _get_nc():
    global _NC_CACHE
    if _NC_CACHE is None:
        _NC_CACHE = _build_kernel()
    return _NC_CACHE


def _split_f32r(a):
    """fp32 = hi + lo with both exactly representable in f32r (12-bit
    significand): hi truncates the mantissa's low 12 bits, lo keeps them."""
    bits = a.view(np.uint32)
    hi = (bits & np.uint32(0xFFFFF000)).view(np.float32)
    lo = a - hi
    return hi, lo


def kernel(x, y, keys, values, W, b):
    x = np.asarray(x, dtype=np.float32)
    y = np.asarray(y)
    keys = np.asarray(keys, dtype=np.float32)
    values = np.asarray(values)
    W = np.asarray(W, dtype=np.float32)
    b = np.asarray(b, dtype=np.float32)

    xT = np.ascontiguousarray(x.T)
    WT = np.ascontiguousarray(W.T)
    bias_bc = np.broadcast_to(b, (128, D)).copy()

    in_maps = []
    for c in range(N_CORES):
        shard = keys[c * N_LOCAL:(c + 1) * N_LOCAL, :]
        keysT = np.ascontiguousarray(shard.T)
        khi, _ = _split_f32r(keysT)
        in_maps.append({
            "xT": xT, "WT": WT, "bias_bc": bias_bc, "keysT_hi": khi,
        })

    nc = _get_nc()
    results = run_bass_kernel_spmd(nc, in_maps, list(range(N_CORES))).results

    # ---- all-gather candidates & re-select global top-k on host ----
    n_cand = NCH * 8                       # 256 per core per row
    vals = np.empty((B, N_CORES * n_cand), np.float32)
    gidx = np.empty((B, N_CORES * n_cand), np.int64)
    chunk_base = (np.arange(NCH, dtype=np.int64)[:, None] * CHUNK_W)  # [NCH, 1]
    for c in range(N_CORES):
        v = results[c]["vals"].reshape(B, NCH, 8)
        ix = results[c]["idxs"].reshape(B, NCH, 8).astype(np.int64)
        g = c * N_LOCAL + chunk_base[None, :, :] + ix
        vals[:, c * n_cand:(c + 1) * n_cand] = v.reshape(B, n_cand)
        gidx[:, c * n_cand:(c + 1) * n_cand] = g.reshape(B, n_cand)

    # top-256 per row; ties resolved toward lower global index like lax.top_k.
    part = np.argpartition(-vals, TOP_K - 1, axis=1)[:, :TOP_K]
    pv = np.take_along_axis(vals, part, axis=1)
    pg = np.take_along_axis(gidx, part, axis=1)
    # order by (-value, index): sort by index first (stable), then by -value
    o1 = np.argsort(pg, axis=1, kind="stable")
    pv = np.take_along_axis(pv, o1, axis=1)
    pg = np.take_along_axis(pg, o1, axis=1)
    o2 = np.argsort(-pv, axis=1, kind="stable")
    cos = np.take_along_axis(pv, o2, axis=1)            # [B, 256] sorted desc
    idx = np.take_along_axis(pg, o2, axis=1)            # [B, 256]

    # exact rescore of the leading candidates to pick the true argmax for
    # y_hat (device scores carry ~2e-6 absolute error from the f32r path)
    q = x.astype(np.float64) @ W.astype(np.float64).T + b.astype(np.float64)
    q /= np.linalg.norm(q, axis=1, keepdims=True)
    r = 8
    head_keys = keys[idx[:, :r], :].astype(np.float64)  # [B, r, D]
    head_scores = np.einsum("bd,brd->br", q, head_keys)
    best = np.argmax(head_scores, axis=1)               # exact argmax among top-8
    top1 = idx[np.arange(B), best]
    y_hat = values[top1]                                # [B, 1], dtype preserved

    # softmax over TEMP * cos (float32, matching jax.nn.softmax)
    z = (TEMP * cos).astype(np.float32)
    z = z - z.max(axis=1, keepdims=True)
    ez = np.exp(z)
    softmax_score = ez / ez.sum(axis=1, keepdims=True)

    # memory hinge loss
    topk_vals = values[idx]                             # [B, 256, 1]
    correct = (topk_vals == y[:, None, :]).squeeze(-1).astype(np.float32)
    pos = np.max(cos * correct, axis=1, keepdims=True)
    neg = np.max(cos * (1.0 - correct), axis=1, keepdims=True)
    has_pos = (correct.sum(axis=1) > 0).astype(np.float32)
    pos = pos * has_pos[:, None]
    loss = np.mean(np.maximum(neg - pos + MARGIN, 0.0).astype(np.float32))

    return (y_hat, softmax_score.astype(np.float32), np.float32(loss))
